# revision 17
# baseline (speedup 1.0000x reference)
"""Trainium2 Bass kernel for GIN message passing (nn_Brouwer_predict).

Computation (reference): two GINConv layers (gather + segment_sum + MLP)
followed by a 3-layer MLP head, on N=50000 nodes / E=800000 edges / D=64.

Strategy:
  * Nodes are relabeled (host-side permutation) into a padded rank-major
    table of 53248 rows = 8 cores x 13 supertiles x 512 nodes, balancing
    per-128-node-tile in-degree.
  * Each core owns 6656 nodes; edges are assigned to the core owning dst.
  * Per 128-node tile, in-edges are processed in chunks of 128 edges:
    messages are gathered with dma_gather (GPSIMD SWDGE gather), and
    aggregated with a one-hot matmul on the PE (one-hot built on DVE from
    dst-relative ids; K=edge chunk, accumulated in PSUM).
  * dma_gather indices are int16 (max 32768 rows), so the node table is
    addressed through two overlapping windows: lo=[0,32768), hi=[20480,53248).
    Each edge is assigned a window on the host; edges in the overlap band are
    used to balance lo/hi chunk counts per tile.
  * Gathers are issued on 4 SWDGE queues (num_swdge_queues=4), each
    supertile-side gather split into 4 sub-gathers round-robined across
    queues — per-queue descriptor-ring drain is the gather throughput
    limiter, and 4 queues + splits run the rings in parallel (~2x).
  * Layer-1 messages are gathered from x in fp32 (rows must be a multiple of
    256B), cast to bf16 on-chip; aggregation matmuls run in bf16 with fp32
    PSUM accumulation. The MLP runs in fp32 feature-major (h^T) layout.
  * h1 is written back node-major as bf16 and exchanged with an AllGather;
    layer-2 gathers read the replicated bf16 h1 table.
"""
import heapq

import numpy as np
import ml_dtypes

BF16 = ml_dtypes.bfloat16

# ---- static problem config ----
N = 50000
E = 800000
D = 64
NC = 8            # cores
P = 128           # partitions / tile size
TPS = 4           # tiles per supertile
ST = 13           # supertiles per core
TPC = ST * TPS    # 52 tiles per core
NPC = TPC * P     # 6656 nodes per core
NPAD = NC * NPC   # 53248
LO_BASE, LO_SIZE = 0, 32768
HI_BASE, HI_SIZE = NPAD - 32768, 32768   # 20480..53248


def _set_config(n, e, n_cores, st):
    """Debug hook: reconfigure module-level problem dims (small builds)."""
    global N, E, NC, ST, TPC, NPC, NPAD, LO_BASE, LO_SIZE, HI_BASE, HI_SIZE
    N, E, NC, ST = n, e, n_cores, st
    TPC = ST * TPS
    NPC = TPC * P
    NPAD = NC * NPC
    LO_BASE, LO_SIZE = 0, min(32768, NPAD)
    HI_BASE = max(0, NPAD - 32768)
    HI_SIZE = NPAD - HI_BASE


def _preprocess(edge_tensor):
    """Node relabeling + per-core chunked edge arrays.

    row id layout (rank-major): row = core*NPC + j, j in [0, NPC);
    supertile s = j//512, tile tl = (j%512)//128.
    """
    src = np.asarray(edge_tensor[0], dtype=np.int64)
    dst = np.asarray(edge_tensor[1], dtype=np.int64)

    in_deg = np.bincount(dst, minlength=N)

    ntiles = NC * TPC
    order = np.argsort(-in_deg, kind="stable")
    heap = [(0, t) for t in range(ntiles)]
    heapq.heapify(heap)
    tile_nodes = [[] for _ in range(ntiles)]
    tile_load = [0] * ntiles
    for n in order:
        while True:
            load, t = heapq.heappop(heap)
            if len(tile_nodes[t]) < P:
                break
        tile_nodes[t].append(n)
        tile_load[t] = load + int(in_deg[n])
        if len(tile_nodes[t]) < P:
            heapq.heappush(heap, (tile_load[t], t))

    row_of = np.full(N, -1, dtype=np.int64)
    for t in range(ntiles):
        c, tloc = divmod(t, TPC)
        base = c * NPC + tloc * P
        for k, n in enumerate(tile_nodes[t]):
            row_of[n] = base + k
    assert (row_of >= 0).all()

    src_r = row_of[src]
    dst_r = row_of[dst]
    gtile = (dst_r // NPC) * TPC + (dst_r % NPC) // P

    lo_only = src_r < HI_BASE
    hi_only = src_r >= LO_SIZE
    flex = ~lo_only & ~hi_only

    side = np.zeros(E, dtype=np.int8)
    side[hi_only] = 1
    tile_order = np.argsort(gtile, kind="stable")
    bounds = np.searchsorted(gtile[tile_order], np.arange(ntiles + 1))
    per_tile_edges = [tile_order[bounds[t]:bounds[t + 1]] for t in range(ntiles)]
    for t in range(ntiles):
        e_ids = per_tile_edges[t]
        if len(e_ids) == 0:
            continue
        f = e_ids[flex[e_ids]]
        nlo = int(lo_only[e_ids].sum())
        tot = len(e_ids)
        lo_n = int(np.clip((tot + 1) // 2, nlo, nlo + len(f)))
        side[f[lo_n - nlo:]] = 1

    cnt = np.zeros((ntiles, 2), dtype=np.int64)
    for t in range(ntiles):
        e_ids = per_tile_edges[t]
        cnt[t, 0] = int((side[e_ids] == 0).sum())
        cnt[t, 1] = int((side[e_ids] == 1).sum())
    cnt_pos = cnt.reshape(NC, TPC, 2)
    C = np.ceil(cnt_pos.max(axis=0) / P).astype(np.int64)  # [TPC, 2]

    slots = [int(C[:, sd].sum()) * P for sd in range(2)]   # slots per core/side
    # edge slot arrays in (tile-in-core, chunk, part) order
    idx_arr = [np.zeros((NC, slots[sd]), dtype=np.int16) for sd in range(2)]
    rel_arr = [np.full((NC, slots[sd]), -1.0, dtype=BF16) for sd in range(2)]

    slot_off = np.zeros((TPC, 2), dtype=np.int64)
    for sd in range(2):
        off = 0
        for tloc in range(TPC):
            slot_off[tloc, sd] = off
            off += int(C[tloc, sd]) * P

    base_of_side = (LO_BASE, HI_BASE)
    for t in range(ntiles):
        c, tloc = divmod(t, TPC)
        e_ids = per_tile_edges[t]
        tile_row0 = c * NPC + tloc * P
        for sd in range(2):
            es = e_ids[side[e_ids] == sd]
            es = es[np.argsort(src_r[es], kind="stable")]
            k = len(es)
            o = slot_off[tloc, sd]
            idx_arr[sd][c, o:o + k] = (src_r[es] - base_of_side[sd]).astype(np.int16)
            rel_arr[sd][c, o:o + k] = (dst_r[es] - tile_row0).astype(np.float32).astype(BF16)

    return dict(row_of=row_of, C=C, idx=idx_arr, rel=rel_arr, slot_off=slot_off)


def _wrap_idx(ids):
    """dma_gather index layout: [128, n/16] int16, idx i at [i%16, i//16],
    replicated 8x down the partition dim."""
    n = len(ids)
    assert n % 16 == 0
    w = ids.reshape(n // 16, 16).T            # [16, n/16]
    return np.tile(w, (8, 1))                 # [128, n/16]


def _build_core_inputs(prep, x_tab, weights):
    """Per-core in_maps for run_bass_kernel_spmd."""
    C = prep["C"]
    idx_arr, rel_arr = prep["idx"], prep["rel"]
    # idx arrays -> wrapped per-supertile layout, concatenated on columns
    maps = []
    for c in range(NC):
        m = dict(x_tab=x_tab,
                 x_loc=np.ascontiguousarray(x_tab[c * NPC:(c + 1) * NPC]))
        m.update(weights)
        for sd, nm in ((0, "lo"), (1, "hi")):
            wrapped = []
            rels = []
            off = 0
            for s in range(ST):
                nsl = int(C[s * TPS:(s + 1) * TPS, sd].sum()) * P
                ids = idx_arr[sd][c, off:off + nsl]
                rl = rel_arr[sd][c, off:off + nsl]
                wrapped.append(_wrap_idx(ids))
                rels.append(rl.reshape(-1, P).T)   # [128, chunks]
                off += nsl
            m[f"idx_{nm}"] = np.ascontiguousarray(np.concatenate(wrapped, axis=1))
            m[f"rel_{nm}"] = np.ascontiguousarray(np.concatenate(rels, axis=1))
        maps.append(m)
    return maps


def _build_program(C, stage="full", nq=4, single_packet=False, gsplit=4,
                   bufs=4):
    """Build the Bacc/Tile program. C: [TPC, 2] per-tile chunk counts.
    stage: debug prefix — "l1" (phase A only), "ag" (+AllGather), "full"."""
    import concourse.bacc as bacc
    import concourse.bass as bass
    import concourse.mybir as mybir
    import concourse.tile as tile
    from concourse.masks import make_identity

    f32 = mybir.dt.float32
    bf16 = mybir.dt.bfloat16
    i16 = mybir.dt.int16
    i32 = mybir.dt.int32

    nc = bacc.Bacc("TRN2", target_bir_lowering=False, debug=False,
                   num_devices=NC, num_swdge_queues=nq)

    x_tab = nc.dram_tensor("x_tab", [NPAD, D], f32, kind="ExternalInput")
    x_loc = nc.dram_tensor("x_loc", [NPC, D], f32, kind="ExternalInput")
    w1 = nc.dram_tensor("w1", [D, 128], f32, kind="ExternalInput")
    w2 = nc.dram_tensor("w2", [128, 256], f32, kind="ExternalInput")
    w3 = nc.dram_tensor("w3", [256, 128], f32, kind="ExternalInput")
    w4 = nc.dram_tensor("w4", [128, 128], f32, kind="ExternalInput")
    w5 = nc.dram_tensor("w5", [128, 1], f32, kind="ExternalInput")
    b1 = nc.dram_tensor("b1", [128, 1], f32, kind="ExternalInput")
    b2 = nc.dram_tensor("b2", [256, 1], f32, kind="ExternalInput")
    b3 = nc.dram_tensor("b3", [128, 1], f32, kind="ExternalInput")
    b4 = nc.dram_tensor("b4", [128, 1], f32, kind="ExternalInput")
    b5 = nc.dram_tensor("b5", [1, 1], f32, kind="ExternalInput")

    slots = [int(C[:, sd].sum()) * P for sd in range(2)]
    idx_d = [nc.dram_tensor(f"idx_{nm}", [P, slots[sd] // 16], i16,
                            kind="ExternalInput")
             for sd, nm in ((0, "lo"), (1, "hi"))]
    rel_d = [nc.dram_tensor(f"rel_{nm}", [P, slots[sd] // P], bf16,
                            kind="ExternalInput")
             for sd, nm in ((0, "lo"), (1, "hi"))]

    y_out = nc.dram_tensor("y", [ST, 512], f32, kind="ExternalOutput")

    h1_loc = nc.dram_tensor("h1_loc", [NPC, 128], bf16, kind="Internal")
    h1_full = nc.dram_tensor("h1_full", [NPAD, 128], bf16, kind="Internal",
                             addr_space="Shared" if NC > 4 else "Local")

    # per-supertile chunk counts and running offsets per side
    Cs = C.reshape(ST, TPS, 2)
    st_chunks = [[int(Cs[s, :, sd].sum()) for s in range(ST)] for sd in range(2)]
    st_off = [[0] * (ST + 1) for _ in range(2)]
    for sd in range(2):
        for s in range(ST):
            st_off[sd][s + 1] = st_off[sd][s] + st_chunks[sd][s]

    with tile.TileContext(nc) as tc:
        with (
            tc.tile_pool(name="const", bufs=1) as constp,
            tc.tile_pool(name="wpool", bufs=1) as wpool,
            tc.tile_pool(name="h1res", bufs=1) as h1res,
            tc.tile_pool(name="idxp", bufs=bufs) as idxp,
            tc.tile_pool(name="relp", bufs=bufs) as relp,
            tc.tile_pool(name="gath", bufs=bufs) as gath,
            tc.tile_pool(name="ohp", bufs=bufs) as ohp,
            tc.tile_pool(name="xloc", bufs=2) as xlocp,
            tc.tile_pool(name="mlp", bufs=2) as mlpp,
            tc.tile_pool(name="h1nm", bufs=4) as h1nmp,
            tc.tile_pool(name="psA", bufs=2, space="PSUM") as psA,
            tc.tile_pool(name="psB", bufs=2, space="PSUM") as psB,
        ):
            # ---- constants ----
            ident = constp.tile([P, P], f32)
            make_identity(nc, ident[:, :])
            iota_i = constp.tile([P, P], i32)
            nc.gpsimd.iota(iota_i[:, :], pattern=[[1, P]], base=0,
                           channel_multiplier=0)
            iota_b = constp.tile([P, P], bf16)
            nc.vector.tensor_copy(iota_b[:, :], iota_i[:, :])

            w1s = wpool.tile([D, 128], f32)
            nc.sync.dma_start(w1s[:, :], w1[:, :])
            w2s = wpool.tile([128, 256], f32)
            nc.sync.dma_start(w2s[:, :], w2[:, :])
            w3s = wpool.tile([128, 256], f32)   # two K-halves side by side
            nc.sync.dma_start(w3s[:, 0:128], w3[0:128, :])
            nc.sync.dma_start(w3s[:, 128:256], w3[128:256, :])
            w4s = wpool.tile([128, 128], f32)
            nc.sync.dma_start(w4s[:, :], w4[:, :])
            w5s = wpool.tile([128, 1], f32)
            nc.sync.dma_start(w5s[:, :], w5[:, :])
            b1s = wpool.tile([128, 1], f32)
            nc.sync.dma_start(b1s[:, :], b1[:, :])
            b2s = wpool.tile([128, 2], f32)
            nc.sync.dma_start(b2s[:, 0:1], b2[0:128, :])
            nc.sync.dma_start(b2s[:, 1:2], b2[128:256, :])
            b3s = wpool.tile([128, 1], f32)
            nc.sync.dma_start(b3s[:, :], b3[:, :])
            b4s = wpool.tile([128, 1], f32)
            nc.sync.dma_start(b4s[:, :], b4[:, :])
            b5s = wpool.tile([1, 1], f32)
            nc.sync.dma_start(b5s[:, :], b5[:, :])

            h1T = h1res.tile([P, NPC], f32)     # resident h1^T (feature-major)

            x_lo = x_tab[LO_BASE:LO_BASE + LO_SIZE, :]
            x_hi = x_tab[HI_BASE:HI_BASE + HI_SIZE, :]
            h_lo = h1_full[LO_BASE:LO_BASE + LO_SIZE, :]
            h_hi = h1_full[HI_BASE:HI_BASE + HI_SIZE, :]

            def onehot_ap(iota_ap, nch):
                a = iota_ap
                return bass.AP(a.tensor, a.offset, [a.ap[0], [0, nch], a.ap[1]])

            def gather_phase(s, src_views, elem, gdt):
                """Issue idx/rel loads + gathers for supertile s. Returns
                (g_tiles, rel_tiles) per side."""
                gs, rls = [], []
                for sd in range(2):
                    nch = st_chunks[sd][s]
                    o = st_off[sd][s]
                    idx_sb = idxp.tile([P, nch * 8], i16, tag=f"idx{sd}")
                    nc.sync.dma_start(idx_sb[:, :],
                                      idx_d[sd][:, o * 8:(o + nch) * 8])
                    rel_sb = relp.tile([P, nch], bf16, tag=f"rel{sd}")
                    nc.sync.dma_start(rel_sb[:, :], rel_d[sd][:, o:o + nch])
                    g = gath.tile([P, nch, elem], gdt, tag=f"g{sd}")
                    nsp = max(1, min(gsplit, nch))
                    bnd = [round(i * nch / nsp) for i in range(nsp + 1)]
                    for i in range(nsp):
                        c0, c1 = bnd[i], bnd[i + 1]
                        if c1 == c0:
                            continue
                        nc.gpsimd.dma_gather(
                            g[:, c0:c1, :], src_views[sd],
                            idx_sb[:, c0 * 8:c1 * 8], (c1 - c0) * P,
                            (c1 - c0) * P, elem,
                            single_packet=single_packet,
                            queue_num=((s * 2 + sd) * nsp + i) % nq)
                    gs.append(g)
                    rls.append(rel_sb)
                return gs, rls

            def agg_tile(s, tl, gs, rls, elem, psum_pool, extra_cast=None):
                """One-hot matmul aggregation for tile tl of supertile s.
                Returns the PSUM tile [128, elem] (fp32)."""
                ps = psum_pool.tile([P, elem], f32, space="PSUM", tag="agg")
                nmm = int(Cs[s, tl, 0] + Cs[s, tl, 1])
                k = 0
                for sd in range(2):
                    nch = int(Cs[s, tl, sd])
                    if nch == 0:
                        continue
                    # chunk offset of this tile within the supertile buffers
                    co = int(Cs[s, :tl, sd].sum())
                    oh = ohp.tile([P, nch, P], bf16, tag=f"oh{sd}")
                    nc.vector.tensor_tensor(
                        out=oh[:, :, :],
                        in0=onehot_ap(iota_b[:, :], nch),
                        in1=rls[sd][:, co:co + nch].to_broadcast([P, nch, P]),
                        op=mybir.AluOpType.is_equal)
                    g = gs[sd] if extra_cast is None else extra_cast[sd]
                    for cch in range(nch):
                        nc.tensor.matmul(ps[:, :], oh[:, cch, :],
                                         g[:, co + cch, :],
                                         start=(k == 0), stop=(k == nmm - 1))
                        k += 1
                return ps

            if stage == "agearly":
                # timing-only: issue the AG first so it overlaps phase A
                nc.gpsimd.collective_compute(
                    "AllGather", mybir.AluOpType.bypass,
                    replica_groups=[list(range(NC))],
                    ins=[h1_loc.ap()], outs=[h1_full.ap()])

            # ---------------- Phase A: layer 1 ----------------
            for s in range(ST):
                gs, rls = gather_phase(s, (x_lo, x_hi), D, f32)
                # cast gathered messages to bf16 for the PE
                gbf = []
                for sd in range(2):
                    nch = st_chunks[sd][s]
                    gb = gath.tile([P, nch, D], bf16, tag=f"gb{sd}")
                    nc.vector.tensor_copy(gb[:, :, :], gs[sd][:, :, :])
                    gbf.append(gb)

                if stage == "l1g":
                    for tl in range(TPS):
                        col = (s * TPS + tl) * P
                        h1nm = h1nmp.tile([P, 128], bf16, tag="h1nm")
                        nc.vector.tensor_copy(h1nm[:, 0:D], gbf[0][:, tl, :])
                        nc.vector.tensor_copy(h1nm[:, D:2 * D],
                                              gbf[1][:, tl, :])
                        nc.sync.dma_start(h1_loc[col:col + P, :], h1nm[:, :])
                    nc.scalar.memzero(h1T[:, s * 512:(s + 1) * 512])
                    continue

                hinT = mlpp.tile([D, 512], f32, tag="hinT")
                for tl in range(TPS):
                    row0 = s * 512 + tl * P
                    xs = xlocp.tile([P, D], f32, tag="xs")
                    nc.sync.dma_start(xs[:, :], x_loc[row0:row0 + P, :])
                    ps = agg_tile(s, tl, gbf, rls, D, psB)
                    hin = xlocp.tile([P, D], f32, tag="hin")
                    nc.vector.tensor_add(hin[:, :], ps[:, :], xs[:, :])
                    if stage == "l1agg":
                        col = (s * TPS + tl) * P
                        h1nm = h1nmp.tile([P, 128], bf16, tag="h1nm")
                        nc.vector.tensor_copy(h1nm[:, 0:D], hin[:, :])
                        nc.vector.tensor_copy(h1nm[:, D:2 * D], hin[:, :])
                        nc.sync.dma_start(h1_loc[col:col + P, :], h1nm[:, :])
                        continue
                    pst = psB.tile([D, P], f32, space="PSUM", tag="tr")
                    nc.tensor.transpose(pst[:, :], hin[:, :], ident[:, :])
                    nc.scalar.copy(hinT[:, tl * P:(tl + 1) * P], pst[:, :])
                if stage == "l1agg":
                    nc.scalar.memzero(h1T[:, s * 512:(s + 1) * 512])
                    continue

                ps_h1 = psA.tile([P, 512], f32, space="PSUM", tag="mm")
                nc.tensor.matmul(ps_h1[:, :], w1s[:, :], hinT[:, :],
                                 start=True, stop=True)
                nc.scalar.activation(h1T[:, s * 512:(s + 1) * 512],
                                     ps_h1[:, :],
                                     mybir.ActivationFunctionType.Relu,
                                     bias=b1s[:, :1])
                if stage == "l1mm":
                    continue
                # node-major bf16 h1 for the gather table
                for tl in range(TPS):
                    col = (s * TPS + tl) * P
                    pst2 = psB.tile([P, P], f32, space="PSUM", tag="tr")
                    nc.tensor.transpose(pst2[:, :], h1T[:, col:col + P],
                                        ident[:, :])
                    h1nm = h1nmp.tile([P, 128], bf16, tag="h1nm")
                    nc.vector.tensor_copy(h1nm[:, :], pst2[:, :])
                    nc.sync.dma_start(h1_loc[col:col + P, :], h1nm[:, :])

            # ---------------- AllGather ----------------
            if stage in ("ag", "full") and stage != "noag_":
                nc.gpsimd.collective_compute(
                    "AllGather", mybir.AluOpType.bypass,
                    replica_groups=[list(range(NC))],
                    ins=[h1_loc.ap()], outs=[h1_full.ap()])

            if stage in ("l1", "ag", "l1g", "l1agg", "l1mm", "noag"):
                for s in range(ST):
                    ysb0 = h1nmp.tile([1, 512], f32, tag="ysb")
                    nc.vector.tensor_copy(ysb0[:, :],
                                          h1T[0:1, s * 512:(s + 1) * 512])
                    nc.sync.dma_start(y_out[s:s + 1, :], ysb0[:, :])

            # ---------------- Phase B: layer 2 + head ----------------
            for s in range(ST) if stage in ("full", "noag", "agearly") else []:
                gs, rls = gather_phase(s, (h_lo, h_hi), 128, bf16)

                h2inT = mlpp.tile([P, 512], f32, tag="h2inT")
                for tl in range(TPS):
                    ps = agg_tile(s, tl, gs, rls, 128, psB)
                    agg_sb = xlocp.tile([P, P], f32, tag="agg_sb")
                    nc.vector.tensor_copy(agg_sb[:, :], ps[:, :])
                    pst = psB.tile([P, P], f32, space="PSUM", tag="tr")
                    nc.tensor.transpose(pst[:, :], agg_sb[:, :], ident[:, :])
                    col = (s * TPS + tl) * P
                    nc.vector.tensor_add(h2inT[:, tl * P:(tl + 1) * P],
                                         pst[:, :], h1T[:, col:col + P])

                ps_a = psA.tile([P, 512], f32, space="PSUM", tag="mm")
                nc.tensor.matmul(ps_a[:, :], w2s[:, 0:128], h2inT[:, :],
                                 start=True, stop=True)
                h2Ta = mlpp.tile([P, 512], f32, tag="h2Ta")
                nc.scalar.activation(h2Ta[:, :], ps_a[:, :],
                                     mybir.ActivationFunctionType.Relu,
                                     bias=b2s[:, 0:1])
                ps_b = psA.tile([P, 512], f32, space="PSUM", tag="mm")
                nc.tensor.matmul(ps_b[:, :], w2s[:, 128:256], h2inT[:, :],
                                 start=True, stop=True)
                h2Tb = mlpp.tile([P, 512], f32, tag="h2Tb")
                nc.scalar.activation(h2Tb[:, :], ps_b[:, :],
                                     mybir.ActivationFunctionType.Relu,
                                     bias=b2s[:, 1:2])

                ps_3 = psA.tile([P, 512], f32, space="PSUM", tag="mm")
                nc.tensor.matmul(ps_3[:, :], w3s[:, 0:128], h2Ta[:, :],
                                 start=True, stop=False)
                nc.tensor.matmul(ps_3[:, :], w3s[:, 128:256], h2Tb[:, :],
                                 start=False, stop=True)
                h3T = mlpp.tile([P, 512], f32, tag="h3T")
                nc.scalar.activation(h3T[:, :], ps_3[:, :],
                                     mybir.ActivationFunctionType.Relu,
                                     bias=b3s[:, :1])

                ps_4 = psA.tile([P, 512], f32, space="PSUM", tag="mm")
                nc.tensor.matmul(ps_4[:, :], w4s[:, :], h3T[:, :],
                                 start=True, stop=True)
                h4T = mlpp.tile([P, 512], f32, tag="h4T")
                nc.scalar.activation(h4T[:, :], ps_4[:, :],
                                     mybir.ActivationFunctionType.Relu,
                                     bias=b4s[:, :1])

                ps_y = psA.tile([1, 512], f32, space="PSUM", tag="mm")
                nc.tensor.matmul(ps_y[:, :], w5s[:, :], h4T[:, :],
                                 start=True, stop=True)
                y_sb = h1nmp.tile([1, 512], f32, tag="ysb")
                nc.vector.tensor_scalar(
                    out=y_sb[:, :], in0=ps_y[:, :],
                    scalar1=b5s[:1, :1], scalar2=None,
                    op0=mybir.AluOpType.add)
                nc.sync.dma_start(y_out[s:s + 1, :], y_sb[:, :])

    nc.compile()
    return nc


def _prepare(inputs):
    """Preprocess + build: returns (nc, in_maps, prep)."""
    x = np.asarray(inputs["x"], np.float32)
    edge_tensor = np.asarray(inputs["edge_tensor"])

    prep = _preprocess(edge_tensor)
    row_of = prep["row_of"]

    x_tab = np.zeros((NPAD, D), np.float32)
    x_tab[row_of] = x

    weights = dict(
        w1=np.ascontiguousarray(np.asarray(inputs["W1"], np.float32)),
        w2=np.ascontiguousarray(np.asarray(inputs["W2"], np.float32)),
        w3=np.ascontiguousarray(np.asarray(inputs["W3"], np.float32)),
        w4=np.ascontiguousarray(np.asarray(inputs["W4"], np.float32)),
        w5=np.ascontiguousarray(np.asarray(inputs["W5"], np.float32)),
        b1=np.asarray(inputs["b1"], np.float32).reshape(128, 1),
        b2=np.asarray(inputs["b2"], np.float32).reshape(256, 1),
        b3=np.asarray(inputs["b3"], np.float32).reshape(128, 1),
        b4=np.asarray(inputs["b4"], np.float32).reshape(128, 1),
        b5=np.asarray(inputs["b5"], np.float32).reshape(1, 1),
    )

    in_maps = _build_core_inputs(prep, x_tab, weights)
    nc = _build_program(prep["C"])
    return nc, in_maps, prep


def kernel(**inputs):
    nc, in_maps, prep = _prepare(inputs)
    row_of = prep["row_of"]

    import os
    import jax
    try:
        devs = jax.devices()
    except Exception:
        devs = []
    if not devs or devs[0].platform != "axon":
        jax.config.update("jax_platforms", "axon,cpu")
        try:
            from jax.extend.backend import clear_backends
            clear_backends()
        except Exception:
            pass

    from concourse.bass_utils import run_bass_kernel_spmd
    trace = bool(os.environ.get("KERNEL_TRACE"))
    global LAST_RESULTS
    last_exc = None
    for attempt in range(3):
        try:
            res = run_bass_kernel_spmd(nc, in_maps, core_ids=list(range(NC)),
                                       trace=trace)
            y_pad = np.zeros((NPAD,), np.float32)
            for c in range(NC):
                y_pad[c * NPC:(c + 1) * NPC] = res.results[c]["y"].reshape(NPC)
            y = y_pad[row_of].reshape(N, 1).astype(np.float32)
            # guard against corrupt results from a dying relay connection
            if not np.isfinite(y).all():
                raise RuntimeError("non-finite kernel output")
            LAST_RESULTS = res
            return y
        except Exception as e:
            last_exc = e
            # missing NTFF hook or transient device error: drop trace,
            # reset backends, retry
            trace = False
            try:
                from jax.extend.backend import clear_backends
                clear_backends()
            except Exception:
                pass
    raise last_exc


if __name__ == "__main__":
    import jax
    jax.config.update("jax_platforms", "cpu")
    import reference

    inputs = {k: np.asarray(v) for k, v in reference.setup_inputs().items()}
    y = kernel(**inputs)
    expected = np.asarray(reference.reference(**reference.setup_inputs()))
    err = np.abs(y - expected).max()
    print("abs err:", err, "rel:", err / np.abs(expected).max())



# revision 25
# speedup vs baseline: 4.0083x; 4.0083x over previous
"""Trainium2 Bass kernel for GIN message passing (nn_Brouwer_predict).

Computation (reference): two GINConv layers (gather + segment_sum + MLP)
followed by a 3-layer MLP head, on N=50000 nodes / E=800000 edges / D=64.

Strategy:
  * Nodes are relabeled (host-side permutation) into a padded rank-major
    table of 53248 rows = 8 cores x 13 supertiles x 512 nodes, balancing
    per-128-node-tile in-degree.
  * Each core owns 6656 nodes; edges are assigned to the core owning dst.
  * Per 128-node tile, in-edges are processed in chunks of 128 edges:
    messages are gathered with dma_gather (GPSIMD SWDGE gather), and
    aggregated with a one-hot matmul on the PE (one-hot built on DVE from
    dst-relative ids; K=edge chunk, accumulated in PSUM).
  * dma_gather indices are int16 (max 32768 rows), so the node table is
    addressed through two overlapping windows: lo=[0,32768), hi=[20480,53248).
    Each edge is assigned a window on the host; edges in the overlap band are
    used to balance lo/hi chunk counts per tile.
  * Gathers are issued on 4 SWDGE queues (num_swdge_queues=4), each
    supertile-side gather split into 4 sub-gathers round-robined across
    queues — per-queue descriptor-ring drain is the gather throughput
    limiter, and 4 queues + splits run the rings in parallel (~2x).
  * Layer-1 messages are gathered from x in fp32 (rows must be a multiple of
    256B), cast to bf16 on-chip; aggregation matmuls run in bf16 with fp32
    PSUM accumulation. The MLP runs in fp32 feature-major (h^T) layout.
  * h1 is written back node-major as bf16 and exchanged with an AllGather;
    layer-2 gathers read the replicated bf16 h1 table.
"""
import heapq

import numpy as np
import ml_dtypes

BF16 = ml_dtypes.bfloat16

# ---- static problem config ----
N = 50000
E = 800000
D = 64
NC = 8            # cores
P = 128           # partitions / tile size
TPS = 4           # tiles per supertile
ST = 13           # supertiles per core
TPC = ST * TPS    # 52 tiles per core
NPC = TPC * P     # 6656 nodes per core
NPAD = NC * NPC   # 53248
LO_BASE, LO_SIZE = 0, 32768
HI_BASE, HI_SIZE = NPAD - 32768, 32768   # 20480..53248


def _set_config(n, e, n_cores, st):
    """Debug hook: reconfigure module-level problem dims (small builds)."""
    global N, E, NC, ST, TPC, NPC, NPAD, LO_BASE, LO_SIZE, HI_BASE, HI_SIZE
    N, E, NC, ST = n, e, n_cores, st
    TPC = ST * TPS
    NPC = TPC * P
    NPAD = NC * NPC
    LO_BASE, LO_SIZE = 0, min(32768, NPAD)
    HI_BASE = max(0, NPAD - 32768)
    HI_SIZE = NPAD - HI_BASE


def _preprocess(edge_tensor):
    """Node relabeling + per-core chunked edge arrays.

    row id layout (rank-major): row = core*NPC + j, j in [0, NPC);
    supertile s = j//512, tile tl = (j%512)//128.
    """
    src = np.asarray(edge_tensor[0], dtype=np.int64)
    dst = np.asarray(edge_tensor[1], dtype=np.int64)

    in_deg = np.bincount(dst, minlength=N)

    ntiles = NC * TPC
    order = np.argsort(-in_deg, kind="stable")
    heap = [(0, t) for t in range(ntiles)]
    heapq.heapify(heap)
    tile_nodes = [[] for _ in range(ntiles)]
    tile_load = [0] * ntiles
    for n in order:
        while True:
            load, t = heapq.heappop(heap)
            if len(tile_nodes[t]) < P:
                break
        tile_nodes[t].append(n)
        tile_load[t] = load + int(in_deg[n])
        if len(tile_nodes[t]) < P:
            heapq.heappush(heap, (tile_load[t], t))

    row_of = np.full(N, -1, dtype=np.int64)
    for t in range(ntiles):
        c, tloc = divmod(t, TPC)
        base = c * NPC + tloc * P
        for k, n in enumerate(tile_nodes[t]):
            row_of[n] = base + k
    assert (row_of >= 0).all()

    src_r = row_of[src]
    dst_r = row_of[dst]
    gtile = (dst_r // NPC) * TPC + (dst_r % NPC) // P

    lo_only = src_r < HI_BASE
    hi_only = src_r >= LO_SIZE
    flex = ~lo_only & ~hi_only

    side = np.zeros(E, dtype=np.int8)
    side[hi_only] = 1
    tile_order = np.argsort(gtile, kind="stable")
    bounds = np.searchsorted(gtile[tile_order], np.arange(ntiles + 1))
    per_tile_edges = [tile_order[bounds[t]:bounds[t + 1]] for t in range(ntiles)]
    for t in range(ntiles):
        e_ids = per_tile_edges[t]
        if len(e_ids) == 0:
            continue
        f = e_ids[flex[e_ids]]
        nlo = int(lo_only[e_ids].sum())
        tot = len(e_ids)
        lo_n = int(np.clip((tot + 1) // 2, nlo, nlo + len(f)))
        side[f[lo_n - nlo:]] = 1

    cnt = np.zeros((ntiles, 2), dtype=np.int64)
    for t in range(ntiles):
        e_ids = per_tile_edges[t]
        cnt[t, 0] = int((side[e_ids] == 0).sum())
        cnt[t, 1] = int((side[e_ids] == 1).sum())
    cnt_pos = cnt.reshape(NC, TPC, 2)
    C = np.ceil(cnt_pos.max(axis=0) / P).astype(np.int64)  # [TPC, 2]

    slots = [int(C[:, sd].sum()) * P for sd in range(2)]   # slots per core/side
    # edge slot arrays in (tile-in-core, chunk, part) order
    idx_arr = [np.zeros((NC, slots[sd]), dtype=np.int16) for sd in range(2)]
    rel_arr = [np.full((NC, slots[sd]), -1.0, dtype=BF16) for sd in range(2)]

    slot_off = np.zeros((TPC, 2), dtype=np.int64)
    for sd in range(2):
        off = 0
        for tloc in range(TPC):
            slot_off[tloc, sd] = off
            off += int(C[tloc, sd]) * P

    base_of_side = (LO_BASE, HI_BASE)
    for t in range(ntiles):
        c, tloc = divmod(t, TPC)
        e_ids = per_tile_edges[t]
        tile_row0 = c * NPC + tloc * P
        for sd in range(2):
            es = e_ids[side[e_ids] == sd]
            es = es[np.argsort(src_r[es], kind="stable")]
            k = len(es)
            o = slot_off[tloc, sd]
            idx_arr[sd][c, o:o + k] = (src_r[es] - base_of_side[sd]).astype(np.int16)
            rel_arr[sd][c, o:o + k] = (dst_r[es] - tile_row0).astype(np.float32).astype(BF16)

    return dict(row_of=row_of, C=C, idx=idx_arr, rel=rel_arr, slot_off=slot_off)


def _wrap_idx(ids):
    """dma_gather index layout: [128, n/16] int16, idx i at [i%16, i//16],
    replicated 8x down the partition dim."""
    n = len(ids)
    assert n % 16 == 0
    w = ids.reshape(n // 16, 16).T            # [16, n/16]
    return np.tile(w, (8, 1))                 # [128, n/16]


def _build_core_inputs(prep, x_tab, x_f32, weights):
    """Per-core in_maps for run_bass_kernel_spmd. x_tab: bf16 padded gather
    table [NPAD, 128]; x_f32: fp32 [NPAD, D] for the per-core self term."""
    C = prep["C"]
    idx_arr, rel_arr = prep["idx"], prep["rel"]
    # idx arrays -> wrapped per-supertile layout, concatenated on columns
    maps = []
    for c in range(NC):
        m = dict(x_tab=x_tab,
                 x_loc=np.ascontiguousarray(x_f32[c * NPC:(c + 1) * NPC]))
        m.update(weights)
        for sd, nm in ((0, "lo"), (1, "hi")):
            wrapped = []
            rels = []
            off = 0
            for s in range(ST):
                nsl = int(C[s * TPS:(s + 1) * TPS, sd].sum()) * P
                ids = idx_arr[sd][c, off:off + nsl]
                rl = rel_arr[sd][c, off:off + nsl]
                wrapped.append(_wrap_idx(ids))
                rels.append(rl.reshape(-1, P).T)   # [128, chunks]
                off += nsl
            m[f"idx_{nm}"] = np.ascontiguousarray(np.concatenate(wrapped, axis=1))
            m[f"rel_{nm}"] = np.ascontiguousarray(np.concatenate(rels, axis=1))
        maps.append(m)
    return maps


def _build_program(C, stage="full", nq=4, single_packet=False, gsplit=4,
                   bufs=4, scratch=16384):
    """Build the Bacc/Tile program. C: [TPC, 2] per-tile chunk counts.
    stage: debug prefix — "l1" (phase A only), "ag" (+AllGather), "full"."""
    import concourse.bacc as bacc
    import concourse.bass as bass
    import concourse.mybir as mybir
    import concourse.tile as tile
    from concourse.masks import make_identity

    f32 = mybir.dt.float32
    bf16 = mybir.dt.bfloat16
    i16 = mybir.dt.int16
    i32 = mybir.dt.int32

    nc = bacc.Bacc("TRN2", target_bir_lowering=False, debug=False,
                   num_devices=NC, num_swdge_queues=nq,
                   dynamic_dma_scratch_size=scratch)

    # x table stored bf16, rows padded to 128 cols (=256B, the dma_gather
    # minimum row size); cols [0:D] hold x, the rest is zero.
    x_tab = nc.dram_tensor("x_tab", [NPAD, 128], bf16, kind="ExternalInput")
    x_loc = nc.dram_tensor("x_loc", [NPC, D], f32, kind="ExternalInput")
    w1 = nc.dram_tensor("w1", [D, 128], f32, kind="ExternalInput")
    w2 = nc.dram_tensor("w2", [128, 256], f32, kind="ExternalInput")
    w3 = nc.dram_tensor("w3", [256, 128], f32, kind="ExternalInput")
    w4 = nc.dram_tensor("w4", [128, 128], f32, kind="ExternalInput")
    w5 = nc.dram_tensor("w5", [128, 1], f32, kind="ExternalInput")
    b1 = nc.dram_tensor("b1", [128, 1], f32, kind="ExternalInput")
    b2 = nc.dram_tensor("b2", [256, 1], f32, kind="ExternalInput")
    b3 = nc.dram_tensor("b3", [128, 1], f32, kind="ExternalInput")
    b4 = nc.dram_tensor("b4", [128, 1], f32, kind="ExternalInput")
    b5 = nc.dram_tensor("b5", [1, 1], f32, kind="ExternalInput")

    slots = [int(C[:, sd].sum()) * P for sd in range(2)]
    idx_d = [nc.dram_tensor(f"idx_{nm}", [P, slots[sd] // 16], i16,
                            kind="ExternalInput")
             for sd, nm in ((0, "lo"), (1, "hi"))]
    rel_d = [nc.dram_tensor(f"rel_{nm}", [P, slots[sd] // P], bf16,
                            kind="ExternalInput")
             for sd, nm in ((0, "lo"), (1, "hi"))]

    y_out = nc.dram_tensor("y", [ST, 512], f32, kind="ExternalOutput")

    h1_loc = nc.dram_tensor("h1_loc", [NPC, 128], bf16, kind="Internal")
    h1_full = nc.dram_tensor("h1_full", [NPAD, 128], bf16, kind="Internal",
                             addr_space="Shared" if NC > 4 else "Local")

    # per-supertile chunk counts and running offsets per side
    Cs = C.reshape(ST, TPS, 2)
    st_chunks = [[int(Cs[s, :, sd].sum()) for s in range(ST)] for sd in range(2)]
    st_off = [[0] * (ST + 1) for _ in range(2)]
    for sd in range(2):
        for s in range(ST):
            st_off[sd][s + 1] = st_off[sd][s] + st_chunks[sd][s]

    with tile.TileContext(nc) as tc:
        with (
            tc.tile_pool(name="const", bufs=1) as constp,
            tc.tile_pool(name="wpool", bufs=1) as wpool,
            tc.tile_pool(name="h1res", bufs=1) as h1res,
            tc.tile_pool(name="idxp", bufs=bufs) as idxp,
            tc.tile_pool(name="relp", bufs=bufs) as relp,
            tc.tile_pool(name="gath", bufs=bufs) as gath,
            tc.tile_pool(name="ohp", bufs=bufs) as ohp,
            tc.tile_pool(name="xloc", bufs=2) as xlocp,
            tc.tile_pool(name="mlp", bufs=2) as mlpp,
            tc.tile_pool(name="h1nm", bufs=4) as h1nmp,
            tc.tile_pool(name="psA", bufs=2, space="PSUM") as psA,
            tc.tile_pool(name="psB", bufs=2, space="PSUM") as psB,
        ):
            # ---- constants ----
            ident = constp.tile([P, P], f32)
            make_identity(nc, ident[:, :])
            iota_i = constp.tile([P, P], i32)
            nc.gpsimd.iota(iota_i[:, :], pattern=[[1, P]], base=0,
                           channel_multiplier=0)
            iota_b = constp.tile([P, P], bf16)
            nc.vector.tensor_copy(iota_b[:, :], iota_i[:, :])

            w1s = wpool.tile([D, 128], f32)
            nc.sync.dma_start(w1s[:, :], w1[:, :])
            w2s = wpool.tile([128, 256], f32)
            nc.sync.dma_start(w2s[:, :], w2[:, :])
            w3s = wpool.tile([128, 256], f32)   # two K-halves side by side
            nc.sync.dma_start(w3s[:, 0:128], w3[0:128, :])
            nc.sync.dma_start(w3s[:, 128:256], w3[128:256, :])
            w4s = wpool.tile([128, 128], f32)
            nc.sync.dma_start(w4s[:, :], w4[:, :])
            w5s = wpool.tile([128, 1], f32)
            nc.sync.dma_start(w5s[:, :], w5[:, :])
            b1s = wpool.tile([128, 1], f32)
            nc.sync.dma_start(b1s[:, :], b1[:, :])
            b2s = wpool.tile([128, 2], f32)
            nc.sync.dma_start(b2s[:, 0:1], b2[0:128, :])
            nc.sync.dma_start(b2s[:, 1:2], b2[128:256, :])
            b3s = wpool.tile([128, 1], f32)
            nc.sync.dma_start(b3s[:, :], b3[:, :])
            b4s = wpool.tile([128, 1], f32)
            nc.sync.dma_start(b4s[:, :], b4[:, :])
            b5s = wpool.tile([1, 1], f32)
            nc.sync.dma_start(b5s[:, :], b5[:, :])

            h1T = h1res.tile([P, NPC], f32)     # resident h1^T (feature-major)

            x_lo = x_tab[LO_BASE:LO_BASE + LO_SIZE, :]
            x_hi = x_tab[HI_BASE:HI_BASE + HI_SIZE, :]
            h_lo = h1_full[LO_BASE:LO_BASE + LO_SIZE, :]
            h_hi = h1_full[HI_BASE:HI_BASE + HI_SIZE, :]

            def onehot_ap(iota_ap, nch):
                a = iota_ap
                return bass.AP(a.tensor, a.offset, [a.ap[0], [0, nch], a.ap[1]])

            def gather_phase(s, src_views, elem, gdt):
                """Issue idx/rel loads + gathers for supertile s. Returns
                (g_tiles, rel_tiles) per side."""
                gs, rls = [], []
                for sd in range(2):
                    nch = st_chunks[sd][s]
                    o = st_off[sd][s]
                    idx_sb = idxp.tile([P, nch * 8], i16, tag=f"idx{sd}")
                    nc.sync.dma_start(idx_sb[:, :],
                                      idx_d[sd][:, o * 8:(o + nch) * 8])
                    rel_sb = relp.tile([P, nch], bf16, tag=f"rel{sd}")
                    nc.sync.dma_start(rel_sb[:, :], rel_d[sd][:, o:o + nch])
                    g = gath.tile([P, nch, elem], gdt, tag=f"g{sd}")
                    nsp = max(1, min(gsplit, nch))
                    bnd = [round(i * nch / nsp) for i in range(nsp + 1)]
                    for i in range(nsp):
                        c0, c1 = bnd[i], bnd[i + 1]
                        if c1 == c0:
                            continue
                        nc.gpsimd.dma_gather(
                            g[:, c0:c1, :], src_views[sd],
                            idx_sb[:, c0 * 8:c1 * 8], (c1 - c0) * P,
                            (c1 - c0) * P, elem,
                            single_packet=single_packet,
                            queue_num=((s * 2 + sd) * nsp + i) % nq)
                    gs.append(g)
                    rls.append(rel_sb)
                return gs, rls

            def agg_tile(s, tl, gs, rls, elem, psum_pool, gcols=None):
                """One-hot matmul aggregation for tile tl of supertile s.
                Returns the PSUM tile [128, elem] (fp32). gcols: use only the
                first gcols columns of the gathered rows (padded-row tables)."""
                ps = psum_pool.tile([P, elem], f32, space="PSUM", tag="agg")
                nmm = int(Cs[s, tl, 0] + Cs[s, tl, 1])
                k = 0
                for sd in range(2):
                    nch = int(Cs[s, tl, sd])
                    if nch == 0:
                        continue
                    # chunk offset of this tile within the supertile buffers
                    co = int(Cs[s, :tl, sd].sum())
                    oh = ohp.tile([P, nch, P], bf16, tag=f"oh{sd}")
                    nc.vector.tensor_tensor(
                        out=oh[:, :, :],
                        in0=onehot_ap(iota_b[:, :], nch),
                        in1=rls[sd][:, co:co + nch].to_broadcast([P, nch, P]),
                        op=mybir.AluOpType.is_equal)
                    g = gs[sd]
                    for cch in range(nch):
                        rhs = (g[:, co + cch, :] if gcols is None
                               else g[:, co + cch, 0:gcols])
                        nc.tensor.matmul(ps[:, :], oh[:, cch, :], rhs,
                                         start=(k == 0), stop=(k == nmm - 1))
                        k += 1
                return ps

            if stage == "agearly":
                # timing-only: issue the AG first so it overlaps phase A
                nc.gpsimd.collective_compute(
                    "AllGather", mybir.AluOpType.bypass,
                    replica_groups=[list(range(NC))],
                    ins=[h1_loc.ap()], outs=[h1_full.ap()])

            # ---------------- Phase A: layer 1 ----------------
            for s in range(ST):
                gs, rls = gather_phase(s, (x_lo, x_hi), 128, bf16)

                if stage == "l1g":
                    for tl in range(TPS):
                        col = (s * TPS + tl) * P
                        h1nm = h1nmp.tile([P, 128], bf16, tag="h1nm")
                        nc.vector.tensor_copy(h1nm[:, 0:D],
                                              gs[0][:, tl, 0:D])
                        nc.vector.tensor_copy(h1nm[:, D:2 * D],
                                              gs[1][:, tl, 0:D])
                        nc.sync.dma_start(h1_loc[col:col + P, :], h1nm[:, :])
                    nc.scalar.memzero(h1T[:, s * 512:(s + 1) * 512])
                    continue

                hinT = mlpp.tile([D, 512], f32, tag="hinT")
                for tl in range(TPS):
                    row0 = s * 512 + tl * P
                    xs = xlocp.tile([P, D], f32, tag="xs")
                    nc.sync.dma_start(xs[:, :], x_loc[row0:row0 + P, :])
                    ps = agg_tile(s, tl, gs, rls, D, psB, gcols=D)
                    hin = xlocp.tile([P, D], f32, tag="hin")
                    nc.vector.tensor_add(hin[:, :], ps[:, :], xs[:, :])
                    if stage == "l1agg":
                        col = (s * TPS + tl) * P
                        h1nm = h1nmp.tile([P, 128], bf16, tag="h1nm")
                        nc.vector.tensor_copy(h1nm[:, 0:D], hin[:, :])
                        nc.vector.tensor_copy(h1nm[:, D:2 * D], hin[:, :])
                        nc.sync.dma_start(h1_loc[col:col + P, :], h1nm[:, :])
                        continue
                    pst = psB.tile([D, P], f32, space="PSUM", tag="tr")
                    nc.tensor.transpose(pst[:, :], hin[:, :], ident[:, :])
                    nc.scalar.copy(hinT[:, tl * P:(tl + 1) * P], pst[:, :])
                if stage == "l1agg":
                    nc.scalar.memzero(h1T[:, s * 512:(s + 1) * 512])
                    continue

                ps_h1 = psA.tile([P, 512], f32, space="PSUM", tag="mm")
                nc.tensor.matmul(ps_h1[:, :], w1s[:, :], hinT[:, :],
                                 start=True, stop=True)
                nc.scalar.activation(h1T[:, s * 512:(s + 1) * 512],
                                     ps_h1[:, :],
                                     mybir.ActivationFunctionType.Relu,
                                     bias=b1s[:, :1])
                if stage == "l1mm":
                    continue
                # node-major bf16 h1 for the gather table
                for tl in range(TPS):
                    col = (s * TPS + tl) * P
                    pst2 = psB.tile([P, P], f32, space="PSUM", tag="tr")
                    nc.tensor.transpose(pst2[:, :], h1T[:, col:col + P],
                                        ident[:, :])
                    h1nm = h1nmp.tile([P, 128], bf16, tag="h1nm")
                    nc.vector.tensor_copy(h1nm[:, :], pst2[:, :])
                    nc.sync.dma_start(h1_loc[col:col + P, :], h1nm[:, :])

            # ---------------- AllGather ----------------
            if stage in ("ag", "full") and stage != "noag_":
                nc.gpsimd.collective_compute(
                    "AllGather", mybir.AluOpType.bypass,
                    replica_groups=[list(range(NC))],
                    ins=[h1_loc.ap()], outs=[h1_full.ap()])

            if stage in ("l1", "ag", "l1g", "l1agg", "l1mm", "noag"):
                for s in range(ST):
                    ysb0 = h1nmp.tile([1, 512], f32, tag="ysb")
                    nc.vector.tensor_copy(ysb0[:, :],
                                          h1T[0:1, s * 512:(s + 1) * 512])
                    nc.sync.dma_start(y_out[s:s + 1, :], ysb0[:, :])

            # ---------------- Phase B: layer 2 + head ----------------
            for s in range(ST) if stage in ("full", "noag", "agearly") else []:
                gs, rls = gather_phase(s, (h_lo, h_hi), 128, bf16)

                h2inT = mlpp.tile([P, 512], f32, tag="h2inT")
                for tl in range(TPS):
                    ps = agg_tile(s, tl, gs, rls, 128, psB)
                    agg_sb = xlocp.tile([P, P], f32, tag="agg_sb")
                    nc.vector.tensor_copy(agg_sb[:, :], ps[:, :])
                    pst = psB.tile([P, P], f32, space="PSUM", tag="tr")
                    nc.tensor.transpose(pst[:, :], agg_sb[:, :], ident[:, :])
                    col = (s * TPS + tl) * P
                    nc.vector.tensor_add(h2inT[:, tl * P:(tl + 1) * P],
                                         pst[:, :], h1T[:, col:col + P])

                ps_a = psA.tile([P, 512], f32, space="PSUM", tag="mm")
                nc.tensor.matmul(ps_a[:, :], w2s[:, 0:128], h2inT[:, :],
                                 start=True, stop=True)
                h2Ta = mlpp.tile([P, 512], f32, tag="h2Ta")
                nc.scalar.activation(h2Ta[:, :], ps_a[:, :],
                                     mybir.ActivationFunctionType.Relu,
                                     bias=b2s[:, 0:1])
                ps_b = psA.tile([P, 512], f32, space="PSUM", tag="mm")
                nc.tensor.matmul(ps_b[:, :], w2s[:, 128:256], h2inT[:, :],
                                 start=True, stop=True)
                h2Tb = mlpp.tile([P, 512], f32, tag="h2Tb")
                nc.scalar.activation(h2Tb[:, :], ps_b[:, :],
                                     mybir.ActivationFunctionType.Relu,
                                     bias=b2s[:, 1:2])

                ps_3 = psA.tile([P, 512], f32, space="PSUM", tag="mm")
                nc.tensor.matmul(ps_3[:, :], w3s[:, 0:128], h2Ta[:, :],
                                 start=True, stop=False)
                nc.tensor.matmul(ps_3[:, :], w3s[:, 128:256], h2Tb[:, :],
                                 start=False, stop=True)
                h3T = mlpp.tile([P, 512], f32, tag="h3T")
                nc.scalar.activation(h3T[:, :], ps_3[:, :],
                                     mybir.ActivationFunctionType.Relu,
                                     bias=b3s[:, :1])

                ps_4 = psA.tile([P, 512], f32, space="PSUM", tag="mm")
                nc.tensor.matmul(ps_4[:, :], w4s[:, :], h3T[:, :],
                                 start=True, stop=True)
                h4T = mlpp.tile([P, 512], f32, tag="h4T")
                nc.scalar.activation(h4T[:, :], ps_4[:, :],
                                     mybir.ActivationFunctionType.Relu,
                                     bias=b4s[:, :1])

                ps_y = psA.tile([1, 512], f32, space="PSUM", tag="mm")
                nc.tensor.matmul(ps_y[:, :], w5s[:, :], h4T[:, :],
                                 start=True, stop=True)
                y_sb = h1nmp.tile([1, 512], f32, tag="ysb")
                nc.vector.tensor_scalar(
                    out=y_sb[:, :], in0=ps_y[:, :],
                    scalar1=b5s[:1, :1], scalar2=None,
                    op0=mybir.AluOpType.add)
                nc.sync.dma_start(y_out[s:s + 1, :], y_sb[:, :])

    nc.compile()
    return nc


def _prepare(inputs):
    """Preprocess + build: returns (nc, in_maps, prep)."""
    x = np.asarray(inputs["x"], np.float32)
    edge_tensor = np.asarray(inputs["edge_tensor"])

    prep = _preprocess(edge_tensor)
    row_of = prep["row_of"]

    x_f32 = np.zeros((NPAD, D), np.float32)
    x_f32[row_of] = x
    x_tab = np.zeros((NPAD, 128), BF16)
    x_tab[:, :D] = x_f32.astype(BF16)

    weights = dict(
        w1=np.ascontiguousarray(np.asarray(inputs["W1"], np.float32)),
        w2=np.ascontiguousarray(np.asarray(inputs["W2"], np.float32)),
        w3=np.ascontiguousarray(np.asarray(inputs["W3"], np.float32)),
        w4=np.ascontiguousarray(np.asarray(inputs["W4"], np.float32)),
        w5=np.ascontiguousarray(np.asarray(inputs["W5"], np.float32)),
        b1=np.asarray(inputs["b1"], np.float32).reshape(128, 1),
        b2=np.asarray(inputs["b2"], np.float32).reshape(256, 1),
        b3=np.asarray(inputs["b3"], np.float32).reshape(128, 1),
        b4=np.asarray(inputs["b4"], np.float32).reshape(128, 1),
        b5=np.asarray(inputs["b5"], np.float32).reshape(1, 1),
    )

    in_maps = _build_core_inputs(prep, x_tab, x_f32, weights)
    nc = _build_program(prep["C"])
    return nc, in_maps, prep


def kernel(**inputs):
    nc, in_maps, prep = _prepare(inputs)
    row_of = prep["row_of"]

    import os
    import jax
    try:
        devs = jax.devices()
    except Exception:
        devs = []
    if not devs or devs[0].platform != "axon":
        jax.config.update("jax_platforms", "axon,cpu")
        try:
            from jax.extend.backend import clear_backends
            clear_backends()
        except Exception:
            pass

    from concourse.bass_utils import run_bass_kernel_spmd
    trace = bool(os.environ.get("KERNEL_TRACE"))
    global LAST_RESULTS
    last_exc = None
    for attempt in range(3):
        try:
            res = run_bass_kernel_spmd(nc, in_maps, core_ids=list(range(NC)),
                                       trace=trace)
            y_pad = np.zeros((NPAD,), np.float32)
            for c in range(NC):
                y_pad[c * NPC:(c + 1) * NPC] = res.results[c]["y"].reshape(NPC)
            y = y_pad[row_of].reshape(N, 1).astype(np.float32)
            # guard against corrupt results from a dying relay connection
            if not np.isfinite(y).all():
                raise RuntimeError("non-finite kernel output")
            LAST_RESULTS = res
            return y
        except Exception as e:
            last_exc = e
            # missing NTFF hook or transient device error: drop trace,
            # reset backends, retry
            trace = False
            try:
                from jax.extend.backend import clear_backends
                clear_backends()
            except Exception:
                pass
    raise last_exc


if __name__ == "__main__":
    import jax
    jax.config.update("jax_platforms", "cpu")
    import reference

    inputs = {k: np.asarray(v) for k, v in reference.setup_inputs().items()}
    y = kernel(**inputs)
    expected = np.asarray(reference.reference(**reference.setup_inputs()))
    err = np.abs(y - expected).max()
    print("abs err:", err, "rel:", err / np.abs(expected).max())

